# revision 1
# baseline (speedup 1.0000x reference)
"""Trainium2 Bass kernel for a 2-layer LSTM (B=256, T=512, D=64, H=512) + FC on last step.

Sharding: data-parallel over batch — 32 samples per NeuronCore on 8 cores.

V3 (default): col-tiled stacked-gate design, ~2.3x faster than V2.
  - All four gate chunks of a layer computed CONCURRENTLY on the PE via
    tile_position col-tiling (4 tiles of M=32 at cols 0/32/64/96) into one
    [128, 512] psum bank; quadrant-pair layout pairs (i|g) on partitions
    0:64 and (f|o) on 64:128, halves of H side by side in the free dim.
  - tanh folded into the single fused sigmoid: g-quadrant weights x2 so
    sigma gives sigma(2g); tanh(g) = 2 sigma(2g) - 1 decoded by DVE STT ops.
    Cell kept as c' = c/2, h kept as h' = h/2 with all h-consuming weights
    pre-doubled; tanh(c) = 2 sigma(4c') - 1.
  - f16 everywhere off-psum (2x DVE modes); one sigmoid instr per layer-step
    plus one for tanh(c).
  - layer1 runs one step behind layer0; x/bias projections primed into
    double-buffered psum banks off the critical path.
V2 (LSTM_KERNEL_VERSION=2): bf16, 4 separate psum banks, per-gate
  activations, block-batched layer-1 input projection.
"""

import numpy as np
import ml_dtypes

import concourse.bass as bass
import concourse.mybir as mybir
import concourse.tile as tile
from concourse.bass_utils import run_bass_kernel_spmd
from concourse.masks import make_identity

BF16 = mybir.dt.bfloat16
F32 = mybir.dt.float32

B, T, D, H, O = 256, 512, 64, 512, 1
G = 4 * H  # 2048
NCORES = 8
BL = B // NCORES  # 32
NK_H = H // 128  # 4 K-chunks for an H-sized contraction
NN = G // 512  # 4 N-chunks of 512 gate columns
SIG = mybir.ActivationFunctionType.Sigmoid
TANH = mybir.ActivationFunctionType.Tanh


def _split_excess_waits(nc, max_waits: int = 1) -> int:
    """This container's walrus rejects >1 sync wait per instruction; move
    excess waits onto preceding same-engine NOPs (same-engine earlier wait
    is ordering-equivalent)."""
    n_split = 0
    for f in nc.m.functions:
        for bb in f.blocks:
            new_insts = []
            for inst in bb.instructions:
                si = inst.sync_info
                if si is not None and si.on_wait and len(si.on_wait) > max_waits:
                    waits = list(si.on_wait)
                    while len(waits) > max_waits:
                        chunk, waits = waits[:max_waits], waits[max_waits:]
                        nop = mybir.InstNoOp(
                            name=f"{inst.name}-wsplit-{n_split}", ins=[], outs=[]
                        )
                        nop.engine = inst.engine
                        nop.sync_info = mybir.SyncInfo(on_wait=chunk, on_update=[])
                        new_insts.append(nop)
                        n_split += 1
                    si.on_wait = waits
                new_insts.append(inst)
            bb.instructions[:] = new_insts
    return n_split


BLOCK_S = 32  # steps per layer-1 input-projection batch (V2 path)


def _default_version() -> int:
    import os

    return int(os.environ.get("LSTM_KERNEL_VERSION", "3"))


def build_lstm_nc(t_steps: int = T, version: int | None = None):
    if version is None:
        version = _default_version()
    if version == 3:
        return build_lstm_nc_v3(t_steps)
    nc = bass.Bass("TRN2")

    xt_d = nc.dram_tensor("xt", [D + 1, t_steps, BL], BF16, kind="ExternalInput")
    w0a_d = nc.dram_tensor("w0a", [D + 1, G], BF16, kind="ExternalInput")
    w0b_d = nc.dram_tensor("w0b", [128, NK_H, G], BF16, kind="ExternalInput")
    w1_d = nc.dram_tensor("w1", [128, 2 * NK_H, G], BF16, kind="ExternalInput")
    w1bias_d = nc.dram_tensor("w1bias", [1, G], BF16, kind="ExternalInput")
    fcw_d = nc.dram_tensor("fcw", [128, NK_H], BF16, kind="ExternalInput")
    ident4_d = nc.dram_tensor("ident4", [128, BL], BF16, kind="ExternalInput")
    fcb_d = nc.dram_tensor("fcb", [1, 1], F32, kind="ExternalInput")
    y_d = nc.dram_tensor("y", [BL, O], F32, kind="ExternalOutput")

    with tile.TileContext(nc) as tc:
        with (
            tc.tile_pool(name="singles", bufs=1) as singles,
            tc.tile_pool(name="state", bufs=1) as state,
            tc.tile_pool(name="work", bufs=3) as work,
            tc.tile_pool(name="psum", bufs=8, space="PSUM") as psum,
        ):
            # --- resident constants ---
            xt_s = singles.tile([D + 1, t_steps, BL], BF16)
            nc.sync.dma_start(out=xt_s, in_=xt_d[:, :, :])
            w0a_s = singles.tile([D + 1, G], BF16)
            nc.sync.dma_start(out=w0a_s, in_=w0a_d[:, :])
            w0b_s = singles.tile([128, NK_H, G], BF16)
            nc.sync.dma_start(out=w0b_s, in_=w0b_d[:, :, :])
            w1_s = singles.tile([128, 2 * NK_H, G], BF16)
            nc.sync.dma_start(out=w1_s, in_=w1_d[:, :, :])
            w1b_s = singles.tile([1, G], BF16)
            nc.sync.dma_start(out=w1b_s, in_=w1bias_d[:, :])
            fcw_s = singles.tile([128, NK_H], BF16)
            nc.sync.dma_start(out=fcw_s, in_=fcw_d[:, :])
            fcb_s = singles.tile([BL, 1], F32)
            nc.sync.dma_start(out=fcb_s, in_=fcb_d[:, :].to_broadcast((BL, 1)))
            ident = singles.tile([BL, BL], BF16)
            make_identity(nc, ident)
            ones_r = singles.tile([1, BL], BF16)
            nc.vector.memset(ones_r, 1.0)
            ones_r128 = singles.tile([1, 128], BF16)
            nc.vector.memset(ones_r128, 1.0)
            ident4_s = singles.tile([128, BL], BF16)
            nc.sync.dma_start(out=ident4_s, in_=ident4_d[:, :])

            # --- recurrent state ---
            h0T = state.tile([128, NK_H, BL], BF16)
            h1T = state.tile([128, NK_H, BL], BF16)
            c0 = state.tile([BL, H], F32)
            c1 = state.tile([BL, H], F32)
            for st in (h0T, h1T, c0, c1):
                nc.vector.memset(st, 0.0)

            def lstm_step(t, hT, cell, w_ih_first, w_s, kslices):
                """One LSTM cell update in gates-[BL, G]-layout.

                w_ih_first: (lhsT, rhs_tile) for the leading K-chunk
                  (x+ones row for layer0 / ones-row bias for layer1 /
                  identity+xp1 inject for layer1-V2).
                kslices: list of (lhsT_tile, k_index_in_w_s) for the
                  remaining accumulation chunks.
                """
                gch = []
                for n in range(NN):
                    ns = slice(n * 512, (n + 1) * 512)
                    gn = psum.tile([BL, 512], F32, tag="ps")
                    if callable(w_ih_first):
                        lhsT0, rhs0, tpos = w_ih_first(n)
                    else:
                        lhsT0, rhs0, tpos = (
                            w_ih_first[0], w_ih_first[1][:, ns], None)
                    nc.tensor.matmul(
                        gn, lhsT0, rhs0, start=True, stop=False,
                        tile_position=tpos,
                    )
                    for j, (lhsT_k, wk) in enumerate(kslices):
                        nc.tensor.matmul(
                            gn,
                            lhsT_k,
                            w_s[:, wk, ns],
                            start=False,
                            stop=(j == len(kslices) - 1),
                        )
                    gch.append(gn)

                sig_i = work.tile([BL, 512], F32, tag="sig_i")
                sig_f = work.tile([BL, 512], F32, tag="sig_f")
                tanh_g = work.tile([BL, 512], F32, tag="tanh_g")
                sig_o = work.tile([BL, 512], F32, tag="sig_o")
                nc.scalar.activation(sig_i, gch[0], SIG)
                nc.scalar.activation(sig_f, gch[1], SIG)
                nc.scalar.activation(tanh_g, gch[2], TANH)
                nc.scalar.activation(sig_o, gch[3], SIG)

                ig = work.tile([BL, 512], F32, tag="ig")
                nc.vector.tensor_mul(ig, sig_i, tanh_g)
                nc.vector.tensor_mul(cell, cell, sig_f)
                nc.vector.tensor_add(cell, cell, ig)
                tanh_c = work.tile([BL, 512], F32, tag="tanh_c")
                nc.scalar.activation(tanh_c, cell, TANH)
                h_new = work.tile([BL, H], BF16, tag="h_new")
                nc.vector.tensor_mul(h_new, sig_o, tanh_c)

                # transpose h_new [32, 512] -> hT [128, 4, 32]
                tp = psum.tile([128, NK_H, BL], BF16, tag="ps")
                for k in range(NK_H):
                    nc.tensor.transpose(
                        tp[:, k, :], h_new[:, k * 128 : (k + 1) * 128], ident
                    )
                nc.vector.tensor_copy(hT, tp)

            if version == 1:
                for t in range(t_steps):
                    lstm_step(
                        t,
                        h0T,
                        c0,
                        (xt_s[:, t, :], w0a_s),
                        w0b_s,
                        [(h0T[:, k, :], k) for k in range(NK_H)],
                    )
                    lstm_step(
                        t,
                        h1T,
                        c1,
                        (ones_r, w1b_s),
                        w1_s,
                        [(h0T[:, k, :], k) for k in range(NK_H)]
                        + [(h1T[:, k, :], NK_H + k) for k in range(NK_H)],
                    )
            else:
                # V2: per block of BLOCK_S steps — run layer0 alone collecting
                # transposed h0 into a block buffer, bulk-GEMM layer1's input
                # projection at full M=128 PE utilization, then run layer1's
                # recurrence with the projection injected via a K=32 identity
                # matmul.
                SB = BLOCK_S
                assert t_steps % SB == 0 and SB % 4 == 0
                h0blk = state.tile([128, NK_H, SB, BL], BF16)
                xp1blk = state.tile([128, SB // 4, NN, 512], BF16)
                for b in range(t_steps // SB):
                    for s in range(SB):
                        t = b * SB + s
                        prev = (
                            h0T if s == 0
                            else h0blk[:, :, s - 1, :]
                        )
                        lstm_step(
                            t,
                            h0blk[:, :, s, :],
                            c0,
                            (xt_s[:, t, :], w0a_s),
                            w0b_s,
                            [(prev[:, k, :], k) for k in range(NK_H)],
                        )
                    nc.vector.tensor_copy(h0T, h0blk[:, :, SB - 1, :])
                    for m in range(SB // 4):
                        for n in range(NN):
                            ns = slice(n * 512, (n + 1) * 512)
                            xp = psum.tile([128, 512], F32, tag="ps")
                            nc.tensor.matmul(
                                xp, ones_r128, w1b_s[:, ns],
                                start=True, stop=False,
                            )
                            for k in range(NK_H):
                                nc.tensor.matmul(
                                    xp,
                                    h0blk[:, k, 4 * m : 4 * m + 4, :].rearrange(
                                        "p a b -> p (a b)"
                                    ),
                                    w1_s[:, k, ns],
                                    start=False,
                                    stop=(k == NK_H - 1),
                                )
                            nc.vector.tensor_copy(xp1blk[:, m, n, :], xp)
                    for s in range(SB):
                        t = b * SB + s
                        lstm_step(
                            t,
                            h1T,
                            c1,
                            lambda n, s=s: (
                                ident4_s[(s % 4) * BL : (s % 4 + 1) * BL, :],
                                xp1blk[
                                    (s % 4) * BL : (s % 4 + 1) * BL, s // 4, n, :
                                ],
                                ((s % 4) * BL, 0) if s % 4 == 3 else None,
                            ),
                            w1_s,
                            [(h1T[:, k, :], NK_H + k) for k in range(NK_H)],
                        )

            # --- fc on last h1 ---
            fcp = psum.tile([BL, O], F32, tag="ps")
            for k in range(NK_H):
                nc.tensor.matmul(
                    fcp,
                    h1T[:, k, :],
                    fcw_s[:, k : k + 1],
                    start=(k == 0),
                    stop=(k == NK_H - 1),
                )
            y_s = work.tile([BL, O], F32, tag="y")
            nc.vector.tensor_add(y_s, fcp, fcb_s)
            nc.sync.dma_start(out=y_d[:, :], in_=y_s)

    _split_excess_waits(nc)
    return nc


F16 = mybir.dt.float16


def build_lstm_nc_v3(t_steps: int = T, split_waits: bool = True):
    """V3: col-tiled stacked-gate matmuls + fused sigmoid + f16 elementwise.

    Per-core layout (batch BL=32):
      gates psum G [128, 512] f32, quadrant-pair layout:
        partitions [ 0: 32]: i[0:256]   | g2[0:256]     (g2 = 2*ghat, tanh via sigmoid)
        partitions [32: 64]: i[256:512] | g2[256:512]
        partitions [64: 96]: f[0:256]   | o[0:256]
        partitions [96:128]: f[256:512] | o[256:512]
      One sigmoid over all 128 partitions gives S = sigma(G) [128,512] f16.
      Cell state c' = c/2 at [64:128, 0:256] f16 (matches f's partitions):
        ig' = (S_g - 0.5) * S_i            -> psum f32 [0:64]   (= i*g/2)
        c' *= S_f ; c' += ig'              (one psum input per op: legal cross)
        tanh(c) = 2*sigma(4c') - 1:  sc = sigma(4c') f16
        h' = (sc - 0.5) * S_o              (= h/2; h-consumers' weights pre-x2)
      h' [64:128, 256] -> 4 PE transposes -> hT [128, 4, 32] f16.
    """
    MUL = mybir.AluOpType.mult
    SUB = mybir.AluOpType.subtract
    nc = bass.Bass("TRN2")

    xt_d = nc.dram_tensor("xt", [D + 1, t_steps, BL], F16, kind="ExternalInput")
    w0a_d = nc.dram_tensor("w0a", [D + 1, 4, 512], F16, kind="ExternalInput")
    w0b_d = nc.dram_tensor("w0b", [128, NK_H, 4, 512], F16, kind="ExternalInput")
    w1i_d = nc.dram_tensor("w1i", [128, NK_H, 4, 512], F16, kind="ExternalInput")
    w1h_d = nc.dram_tensor("w1h", [128, NK_H, 4, 512], F16, kind="ExternalInput")
    b1_d = nc.dram_tensor("b1", [1, 4, 512], F16, kind="ExternalInput")
    fcw_d = nc.dram_tensor("fcw", [128, NK_H], F16, kind="ExternalInput")
    id128_d = nc.dram_tensor("id128", [128, BL], F16, kind="ExternalInput")
    fcb_d = nc.dram_tensor("fcb", [1, 1], F32, kind="ExternalInput")
    y_d = nc.dram_tensor("y", [BL, O], F32, kind="ExternalOutput")

    with tile.TileContext(nc) as tc:
        with (
            tc.tile_pool(name="singles", bufs=1) as singles,
            tc.tile_pool(name="state", bufs=1) as state,
            tc.tile_pool(name="work", bufs=3) as work,
            tc.tile_pool(name="pstate", bufs=1, space="PSUM") as pstate,
        ):
            # --- resident constants ---
            xt_s = singles.tile([D + 1, t_steps, BL], F16, name="xt_s")
            nc.sync.dma_start(out=xt_s, in_=xt_d[:, :, :])
            w0a_s = singles.tile([D + 1, 4, 512], F16, name="w0a_s")
            nc.sync.dma_start(out=w0a_s, in_=w0a_d[:, :, :])
            w0b_s = singles.tile([128, NK_H, 4, 512], F16, name="w0b_s")
            nc.sync.dma_start(out=w0b_s, in_=w0b_d[:, :, :, :])
            w1i_s = singles.tile([128, NK_H, 4, 512], F16, name="w1i_s")
            nc.sync.dma_start(out=w1i_s, in_=w1i_d[:, :, :, :])
            w1h_s = singles.tile([128, NK_H, 4, 512], F16, name="w1h_s")
            nc.sync.dma_start(out=w1h_s, in_=w1h_d[:, :, :, :])
            b1_s = singles.tile([1, 4, 512], F16, name="b1_s")
            nc.sync.dma_start(out=b1_s, in_=b1_d[:, :, :])
            fcw_s = singles.tile([128, NK_H], F16, name="fcw_s")
            nc.sync.dma_start(out=fcw_s, in_=fcw_d[:, :])
            id128_s = singles.tile([128, BL], F16, name="id128_s")
            nc.sync.dma_start(out=id128_s, in_=id128_d[:, :])
            fcb_s = singles.tile([BL, 1], F32, name="fcb_s")
            nc.sync.dma_start(out=fcb_s, in_=fcb_d[:, :].to_broadcast((BL, 1)))
            ones_r = singles.tile([1, BL], F16, name="ones_r")
            nc.vector.memset(ones_r, 1.0)

            # --- state ---
            h0T = state.tile([128, NK_H, BL], F16, name="h0T")
            h1T = state.tile([128, NK_H, BL], F16, name="h1T")
            c0 = state.tile([128, 256], F16, name="c0")   # used at [64:128]
            c1 = state.tile([128, 256], F16, name="c1")
            for st in (h0T, h1T, c0, c1):
                nc.vector.memset(st, 0.0)

            # psum: double-buffered gate banks per layer + transpose/ig scratch
            G0 = [pstate.tile([128, 512], F32, name=f"G0{p}") for p in range(2)]
            G1 = [pstate.tile([128, 512], F32, name=f"G1{p}") for p in range(2)]
            tp0 = pstate.tile([128, 2, NK_H, BL], F16, name="tp0")
            tp1 = pstate.tile([128, 2, NK_H, BL], F16, name="tp1")

            def gate_mms(bank, lhsT, w4, start, stop):
                """One K-slot: 4 col-tiled matmuls (one per quadrant)."""
                for j in range(4):
                    nc.tensor.matmul(
                        bank[32 * j : 32 * (j + 1), :], lhsT, w4[j],
                        start=start, stop=stop, tile_position=(0, 32 * j),
                        skip_group_check=True,
                    )

            def elementwise(Gb, S, c, tcb, hp, igs, hT, tp_half, li):
                # S = sigma(G) over all 128 partitions (g-quadrant pre-scaled x2)
                nc.scalar.activation(S, Gb, SIG)
                # ig' = (S_g - 0.5) * S_i -> sbuf f16 at base 64 (cross-base
                # OUT is legal; inputs share base 0) so the c-add is all-f16
                nc.vector.scalar_tensor_tensor(
                    igs[64:128, :], S[0:64, 256:512], 0.5, S[0:64, 0:256],
                    SUB, MUL)
                # c' *= S_f   (all sbuf, base 64)
                nc.vector.tensor_mul(c[64:128, :], c[64:128, :], S[64:128, 0:256])
                # c' += ig'
                nc.vector.tensor_add(c[64:128, :], c[64:128, :], igs[64:128, :])
                # sc = sigma(4c')  => tanh(c) = 2*sc - 1
                nc.scalar.activation(tcb[64:128, :], c[64:128, :], SIG, scale=4.0)
                # h' = (sc - 0.5) * S_o (= h/2) as [32, 512] at base 0 (two
                # STTs with cross-base output). All transposes then read at
                # row-group 0 like V2 — transposes at row groups other than 0
                # fault the HW when their LDW addresses do not increase.
                # Interleave: transposes 0,1 only need h' cols 0:256, so they
                # run on PE while DVE computes the second half.
                nc.vector.scalar_tensor_tensor(
                    hp[:, 0:256], tcb[64:96, :], 0.5, S[64:96, 256:512],
                    SUB, MUL)
                for k in (0, 1):
                    nc.tensor.transpose(
                        tp_half[:, k, :], hp[:, 128 * k : 128 * (k + 1)],
                        id128_s[0:BL, :])
                nc.vector.scalar_tensor_tensor(
                    hp[:, 256:512], tcb[96:128, :], 0.5, S[96:128, 256:512],
                    SUB, MUL)
                for k in (2, 3):
                    nc.tensor.transpose(
                        tp_half[:, k, :], hp[:, 128 * k : 128 * (k + 1)],
                        id128_s[0:BL, :])
                nc.vector.tensor_copy(hT, tp_half)

            S0 = work.tile([128, 512], F16, tag="S0", name="S0")
            S1 = work.tile([128, 512], F16, tag="S1", name="S1")
            tc0 = work.tile([128, 256], F16, tag="tc0", name="tc0")
            tc1 = work.tile([128, 256], F16, tag="tc1", name="tc1")
            hp0 = work.tile([BL, 512], F16, tag="hp0", name="hp0")
            hp1 = work.tile([BL, 512], F16, tag="hp1", name="hp1")
            igs0 = work.tile([128, 256], F16, tag="igs0", name="igs0")
            igs1 = work.tile([128, 256], F16, tag="igs1", name="igs1")

            # prime: L0 x-projection and L1 bias for step 0
            gate_mms(G0[0], xt_s[:, 0, :], [w0a_s[:, j, :] for j in range(4)],
                     True, False)
            gate_mms(G1[0], ones_r, [b1_s[:, j, :] for j in range(4)],
                     True, False)

            for t in range(t_steps):
                p = t % 2
                # L0 recurrent part for t
                for k in range(NK_H):
                    gate_mms(G0[p], h0T[:, k, :],
                             [w0b_s[:, k, j, :] for j in range(4)],
                             False, k == NK_H - 1)
                # L1 input+recurrent part for t-1
                if t >= 1:
                    q = (t - 1) % 2
                    for k in range(NK_H):
                        gate_mms(G1[q], h0T[:, k, :],
                                 [w1i_s[:, k, j, :] for j in range(4)],
                                 False, False)
                    for k in range(NK_H):
                        gate_mms(G1[q], h1T[:, k, :],
                                 [w1h_s[:, k, j, :] for j in range(4)],
                                 False, k == NK_H - 1)
                # prime next L0 step's x-projection (different bank, off-chain)
                if t + 1 < t_steps:
                    gate_mms(G0[(t + 1) % 2], xt_s[:, t + 1, :],
                             [w0a_s[:, j, :] for j in range(4)], True, False)
                # L0 elementwise for t -> h0T
                elementwise(G0[p], S0, c0, tc0, hp0, igs0, h0T, tp0[:, p], 0)
                # L1 elementwise for t-1 -> h1T
                if t >= 1:
                    q = (t - 1) % 2
                    elementwise(G1[q], S1, c1, tc1, hp1, igs1, h1T, tp1[:, q], 1)
                # prime L1 bias for step t+1 (after sigma1(t-1) read is queued)
                if t + 1 < t_steps:
                    gate_mms(G1[(t + 1) % 2], ones_r,
                             [b1_s[:, j, :] for j in range(4)], True, False)

            # drain: L1 for t = T-1
            q = (t_steps - 1) % 2
            for k in range(NK_H):
                gate_mms(G1[q], h0T[:, k, :],
                         [w1i_s[:, k, j, :] for j in range(4)], False, False)
            for k in range(NK_H):
                gate_mms(G1[q], h1T[:, k, :],
                         [w1h_s[:, k, j, :] for j in range(4)],
                         False, k == NK_H - 1)
            elementwise(G1[q], S1, c1, tc1, hp1, igs1, h1T, tp1[:, q], 1)

            # --- fc on last h1 (h1T holds h1/2; fcw pre-doubled) ---
            fcp = pstate.tile([BL, O], F32, name="fcp")
            for k in range(NK_H):
                nc.tensor.matmul(
                    fcp, h1T[:, k, :], fcw_s[:, k : k + 1],
                    start=(k == 0), stop=(k == NK_H - 1),
                )
            y_s = work.tile([BL, O], F32, tag="y", name="y_s")
            nc.vector.tensor_add(y_s, fcp, fcb_s)
            nc.sync.dma_start(out=y_d[:, :], in_=y_s)

    if split_waits:
        _split_excess_waits(nc)
    return nc


def _perm_cols(wt, scale_g=2.0):
    """[K, 2048] (i,f,g,o blocks) -> [K, 4, 512] quadrant-pair layout."""
    K = wt.shape[0]
    i, f, g, o = (wt[:, 512 * a : 512 * (a + 1)] for a in range(4))
    g = g * scale_g
    out = np.empty((K, 4, 512), np.float32)
    out[:, 0, 0:256] = i[:, 0:256]
    out[:, 0, 256:512] = g[:, 0:256]
    out[:, 1, 0:256] = i[:, 256:512]
    out[:, 1, 256:512] = g[:, 256:512]
    out[:, 2, 0:256] = f[:, 0:256]
    out[:, 2, 256:512] = o[:, 0:256]
    out[:, 3, 0:256] = f[:, 256:512]
    out[:, 3, 256:512] = o[:, 256:512]
    return out


def prep_inputs_v3(x, w_ih_0, w_hh_0, b_ih_0, b_hh_0, w_ih_1, w_hh_1, b_ih_1,
                   b_hh_1, fc_w, fc_b, t_steps: int = T):
    f16 = np.float16

    # layer0 x-side: [65, 4, 512] with bias row (not h-scaled)
    w0a = _perm_cols(np.concatenate([w_ih_0.T, (b_ih_0 + b_hh_0)[None, :]], 0))
    w0a = w0a.astype(f16)

    def hh_prep(w):  # h-consuming weights x2 (h' = h/2), then quadrant perm
        wp = _perm_cols(2.0 * w.T)  # [512, 4, 512]
        return np.ascontiguousarray(
            wp.reshape(NK_H, 128, 4, 512).transpose(1, 0, 2, 3)).astype(f16)

    w0b = hh_prep(w_hh_0)
    w1i = hh_prep(w_ih_1)
    w1h = hh_prep(w_hh_1)
    b1 = _perm_cols((b_ih_1 + b_hh_1)[None, :]).astype(f16)  # [1, 4, 512]
    fcw = np.ascontiguousarray((2.0 * fc_w).reshape(NK_H, 128).T).astype(f16)
    fcb = fc_b.reshape(1, 1).astype(np.float32)
    id128 = np.concatenate([np.eye(BL, dtype=np.float32)] * 4, 0).astype(f16)

    in_maps = []
    for c in range(NCORES):
        xc = x[c * BL : (c + 1) * BL, :t_steps, :]
        xt = np.transpose(xc, (2, 1, 0))
        xt = np.concatenate([xt, np.ones((1, t_steps, BL), np.float32)], axis=0)
        in_maps.append({
            "xt": np.ascontiguousarray(xt).astype(f16),
            "w0a": w0a, "w0b": w0b, "w1i": w1i, "w1h": w1h, "b1": b1,
            "fcw": fcw, "fcb": fcb, "id128": id128,
        })
    return in_maps


def prep_inputs(x, w_ih_0, w_hh_0, b_ih_0, b_hh_0, w_ih_1, w_hh_1, b_ih_1, b_hh_1,
                fc_w, fc_b, t_steps: int = T, version: int | None = None):
    """Host-side layout prep + sharding. Returns per-core in_maps."""
    if version is None:
        version = _default_version()
    if version == 3:
        return prep_inputs_v3(x, w_ih_0, w_hh_0, b_ih_0, b_hh_0, w_ih_1, w_hh_1,
                              b_ih_1, b_hh_1, fc_w, fc_b, t_steps=t_steps)
    bf = ml_dtypes.bfloat16
    w0a = np.concatenate(
        [w_ih_0.T, (b_ih_0 + b_hh_0)[None, :]], axis=0
    ).astype(bf)  # [65, G]
    w0b = np.ascontiguousarray(
        w_hh_0.T.reshape(NK_H, 128, G).transpose(1, 0, 2)
    ).astype(bf)  # [128, 4, G]
    w1 = np.ascontiguousarray(
        np.concatenate([w_ih_1.T, w_hh_1.T], axis=0)
        .reshape(2 * NK_H, 128, G)
        .transpose(1, 0, 2)
    ).astype(bf)  # [128, 8, G]
    w1bias = (b_ih_1 + b_hh_1)[None, :].astype(bf)  # [1, G]
    fcw = np.ascontiguousarray(fc_w.reshape(NK_H, 128).T).astype(bf)  # [128, 4]
    fcb = fc_b.reshape(1, 1).astype(np.float32)
    ident4 = np.concatenate([np.eye(BL, dtype=np.float32)] * 4, axis=0).astype(bf)

    in_maps = []
    for c in range(NCORES):
        xc = x[c * BL : (c + 1) * BL, :t_steps, :]  # [32, T, 64]
        xt = np.transpose(xc, (2, 1, 0))  # [64, T, 32]
        xt = np.concatenate([xt, np.ones((1, t_steps, BL), np.float32)], axis=0)
        in_maps.append(
            {
                "xt": np.ascontiguousarray(xt).astype(bf),
                "w0a": w0a,
                "w0b": w0b,
                "w1": w1,
                "w1bias": w1bias,
                "fcw": fcw,
                "fcb": fcb,
                "ident4": ident4,
            }
        )
    return in_maps


_NC_CACHE = {}


def kernel(x, w_ih_0, w_hh_0, b_ih_0, b_hh_0, w_ih_1, w_hh_1, b_ih_1, b_hh_1,
           fc_w, fc_b):
    x = np.asarray(x, np.float32)
    args = [np.asarray(a, np.float32) for a in (
        w_ih_0, w_hh_0, b_ih_0, b_hh_0, w_ih_1, w_hh_1, b_ih_1, b_hh_1, fc_w, fc_b)]
    if T not in _NC_CACHE:
        _NC_CACHE[T] = build_lstm_nc(T)
    nc = _NC_CACHE[T]
    in_maps = prep_inputs(x, *args, t_steps=T)
    res = run_bass_kernel_spmd(nc, in_maps, core_ids=list(range(NCORES)))
    return np.concatenate([res.results[c]["y"] for c in range(NCORES)], axis=0)

